# revision 1
# baseline (speedup 1.0000x reference)
"""Block-causal attention (B=4, N=2048, C=1024, H=16, block=128) on 8 TRN2 NeuronCores.

Sharding: core = 2*b + g  (b in 0..3 batches, g in 0..1 head-groups of 8 heads).
Each core:
  - computes q^T,k^T (feature-major) and v (token-major) for its batch/head-group
    from a host-pre-transposed x^T and head-sliced w_qkv  (no duplicated FLOPs),
  - block-causal attention: q-tile i attends to k-tiles 0..i (no masking needed,
    128-token blocks align with tiles),
  - partial out-projection with its 512-row slice of w_proj.
Host sums the two partial projections per batch and adds b_proj.

All matmuls run as float32r (single-pass fp22) on the PE.  Attention uses:
  S^T[j] = k_j^T q  (row-packed pair of heads, K=64 each),
  expT = exp(S^T/8) on ScalarE straight out of PSUM,
  out^T += v_j^T expT and row-sums += ones^T expT (col-packed head pairs),
  normalize with DVE reciprocal+mul.  No transposes anywhere on device.
"""

import numpy as np
import ml_dtypes
from collections import deque
from contextlib import ExitStack

B, N, C, H, HD = 4, 2048, 1024, 16, 64
HPC = 8               # heads per core
F = HPC * HD          # 512 features per core
NCORES = 8
SCALE = float(HD) ** -0.5
NT = N // 128         # 16 token tiles
NCH = 4               # token chunks of 512

_CACHE = {}


def _build():
    import concourse.mybir as mybir
    import concourse.tile as tile
    from concourse import bacc

    f32 = mybir.dt.float32
    f32r = mybir.dt.float32r
    bf16 = mybir.dt.bfloat16
    Exp = mybir.ActivationFunctionType.Exp

    nc = bacc.Bacc("TRN2", target_bir_lowering=False, debug=False,
                   num_devices=NCORES)

    xT = nc.dram_tensor("xT", [C, N], f32r, kind="ExternalInput")
    wq = nc.dram_tensor("wq", [C, F], f32r, kind="ExternalInput")
    wk = nc.dram_tensor("wk", [C, F], f32r, kind="ExternalInput")
    wv = nc.dram_tensor("wv", [C, F], f32r, kind="ExternalInput")
    wp = nc.dram_tensor("wp", [F, C], f32r, kind="ExternalInput")
    ones_d = nc.dram_tensor("ones", [128, 64], bf16, kind="ExternalInput")
    out = nc.dram_tensor("out", [N, C], f32, kind="ExternalOutput")

    with tile.TileContext(nc) as tc, ExitStack() as ctx:
        persist = ctx.enter_context(tc.tile_pool(name="persist", bufs=1))
        xt_pool = ctx.enter_context(tc.tile_pool(name="xt", bufs=2))
        qt_pool = ctx.enter_context(tc.tile_pool(name="qt", bufs=2))
        at_pool = ctx.enter_context(tc.tile_pool(name="attnT", bufs=4))
        exp_pool = ctx.enter_context(tc.tile_pool(name="expT", bufs=4))
        rc_pool = ctx.enter_context(tc.tile_pool(name="recip", bufs=1))
        ost_pool = ctx.enter_context(tc.tile_pool(name="ost", bufs=2))
        ps_mm = ctx.enter_context(tc.tile_pool(name="ps_mm", bufs=2, space="PSUM"))
        ps_s = ctx.enter_context(tc.tile_pool(name="ps_s", bufs=2, space="PSUM"))
        ps_av = ctx.enter_context(tc.tile_pool(name="ps_av", bufs=1, space="PSUM"))
        ps_sum = ctx.enter_context(tc.tile_pool(name="ps_sum", bufs=1, space="PSUM"))

        # ---- persistent weights (DMAs emitted later, after chunk-0 xt) ----
        wq_t = [persist.tile([128, F], f32r, name=f"wq{kk}", tag=f"wq{kk}") for kk in range(8)]
        wk_t = [persist.tile([128, F], f32r, name=f"wk{kk}", tag=f"wk{kk}") for kk in range(8)]
        wv_t = [persist.tile([128, F], f32r, name=f"wv{kk}", tag=f"wv{kk}") for kk in range(8)]
        wp_t = [persist.tile([128, C], f32r, name=f"wp{kk}", tag=f"wp{kk}") for kk in range(4)]
        ones_t = persist.tile([128, 64], bf16, name="ones", tag="ones")

        # persistent k^T (per head-pair per chunk) and v (per token tile)
        kt_t = [[persist.tile([128, 512], f32r, name=f"kT{hp}_{jc}", tag=f"kT{hp}_{jc}")
                 for jc in range(NCH)] for hp in range(4)]
        v_t = [persist.tile([128, F], bf16, name=f"v{t}", tag=f"v{t}") for t in range(NT)]

        def load_weights():
            # spread across engine DMA queues so the loads run in parallel
            # with the chunk-0 xt loads on the sync queue
            for kk in range(8):
                nc.scalar.dma_start(wk_t[kk][:], wk[kk * 128:(kk + 1) * 128, :])
            for kk in range(8):
                nc.gpsimd.dma_start(wv_t[kk][:], wv[kk * 128:(kk + 1) * 128, :])
            for kk in range(8):
                nc.sync.dma_start(wq_t[kk][:], wq[kk * 128:(kk + 1) * 128, :])
            nc.gpsimd.dma_start(ones_t[:], ones_d[:])
            for kk in range(4):
                nc.gpsimd.dma_start(wp_t[kk][:], wp[kk * 128:(kk + 1) * 128, :])

        # qt_state[c] / at_state[c] filled lazily by the emit units below
        qt_state = {c: [] for c in range(NCH)}
        at_state = {c: [] for c in range(NCH)}

        def qkv_units(c):
            """Emission units for QKV of token chunk c (xt DMA + 12 matmul
            groups).  Returned as closures so they can be interleaved into
            the previous chunk's attention to keep the PE dense (HAM)."""
            c0 = c * 512
            xt_c = []

            def load():
                for kk in range(8):
                    xt = xt_pool.tile([128, 512], f32r, name=f"xt{kk}",
                                      tag=f"xt{kk}")
                    nc.sync.dma_start(xt[:],
                                      xT[kk * 128:(kk + 1) * 128, c0:c0 + 512])
                    xt_c.append(xt)

            def q_group(hp):
                def emit():
                    ps = ps_mm.tile([128, 512], f32, name="mm", tag="mm")
                    for kk in range(8):
                        nc.tensor.matmul(ps[:],
                                         wq_t[kk][:, hp * 128:(hp + 1) * 128],
                                         xt_c[kk][:],
                                         start=(kk == 0), stop=(kk == 7))
                    qt = qt_pool.tile([128, 512], f32r, name=f"qT{hp}",
                                      tag=f"qT{hp}")
                    nc.vector.tensor_copy(qt[:], ps[:])
                    qt_state[c].append(qt)
                return emit

            def k_group(hp):
                def emit():
                    ps = ps_mm.tile([128, 512], f32, name="mm", tag="mm")
                    for kk in range(8):
                        nc.tensor.matmul(ps[:],
                                         wk_t[kk][:, hp * 128:(hp + 1) * 128],
                                         xt_c[kk][:],
                                         start=(kk == 0), stop=(kk == 7))
                    nc.vector.tensor_copy(kt_t[hp][c][:], ps[:])
                return emit

            def v_group(tl):
                def emit():
                    t = 4 * c + tl
                    ps = ps_mm.tile([128, 512], f32, name="mm", tag="mm")
                    for kk in range(8):
                        nc.tensor.matmul(ps[:],
                                         xt_c[kk][:, tl * 128:(tl + 1) * 128],
                                         wv_t[kk][:],
                                         start=(kk == 0), stop=(kk == 7))
                    nc.vector.tensor_copy(v_t[t][:], ps[:])
                return emit

            units = [load]
            # k/v first (attention chunk c needs them for all j), q last
            for hp in range(4):
                units.append(k_group(hp))
            for tl in range(4):
                units.append(v_group(tl))
            for hp in range(4):
                units.append(q_group(hp))
            return units

        def proj_units(c):
            """Emission units for the partial projection of chunk c."""
            units = []
            for tl in range(4):
                for n2 in range(2):
                    def emit(tl=tl, n2=n2):
                        t = 4 * c + tl
                        ps = ps_mm.tile([128, 512], f32, name="mm", tag="mm")
                        for kk in range(4):
                            nc.tensor.matmul(
                                ps[:],
                                at_state[c][kk][:, tl * 128:(tl + 1) * 128],
                                wp_t[kk][:, n2 * 512:(n2 + 1) * 512],
                                start=(kk == 0), stop=(kk == 3))
                        ost = ost_pool.tile([128, 512], f32, name="ost",
                                            tag="ost")
                        nc.vector.tensor_copy(ost[:], ps[:])
                        nc.sync.dma_start(
                            out[t * 128:(t + 1) * 128,
                                n2 * 512:(n2 + 1) * 512],
                            ost[:])
                    units.append(emit)
            return units

        # chunk 0's QKV has nothing to hide behind — emit it upfront.
        # xt DMAs go first so compute can start before all weights land.
        units0 = qkv_units(0)
        units0[0]()
        load_weights()
        for u in units0[1:]:
            u()

        def attn_unit(c, hp, fillers, stride=2):
            """Attention for (chunk c, head-pair hp): the j-loop over visible
            k-tiles, with filler matmul groups paced in to keep the PE dense."""
            njt = 4 * c + 4
            qt_c = qt_state[c]
            av = ps_av.tile([128, 512], f32, name="av", tag="av")
            sm = ps_sum.tile([128, 512], f32, name="sum", tag="sum")
            for j in range(njt):
                jd = j - 4 * c
                vco = jd * 128 if jd > 0 else 0
                kt = kt_t[hp][j // 4]
                kc = (j % 4) * 128
                ss = ps_s.tile([128, 1024], f32, name="s", tag="s")
                # S^T: row-packed head pair (K=64 each)
                nc.tensor.matmul(ss[:, vco:512],
                                 kt[0:64, kc:kc + 128],
                                 qt_c[hp][0:64, vco:512],
                                 start=True, stop=True)
                nc.tensor.matmul(ss[:, 512 + vco:1024],
                                 kt[64:128, kc:kc + 128],
                                 qt_c[hp][64:128, vco:512],
                                 start=True, stop=True)
                et = exp_pool.tile([128, 1024], bf16, name="e", tag="e")
                if vco:
                    in3 = ss[:].rearrange("p (b q) -> p b q", b=2)[:, :, vco:512]
                    out3 = et[:].rearrange("p (b q) -> p b q", b=2)[:, :, vco:512]
                    nc.scalar.activation(out3, in3, Exp, scale=SCALE)
                else:
                    nc.scalar.activation(et[:], ss[:], Exp, scale=SCALE)
                first, last = (j == 0), (j == njt - 1)
                # out^T accumulation: col-packed head pair (M=64 each)
                nc.tensor.matmul(av[0:64, vco:512],
                                 v_t[j][:, hp * 128:hp * 128 + 64],
                                 et[:, vco:512],
                                 start=first, stop=last)
                nc.tensor.matmul(av[64:128, vco:512],
                                 v_t[j][:, hp * 128 + 64:hp * 128 + 128],
                                 et[:, 512 + vco:1024],
                                 start=first, stop=last)
                # softmax denominators, replicated across partitions
                nc.tensor.matmul(sm[0:64, vco:512],
                                 ones_t[:, 0:64],
                                 et[:, vco:512],
                                 start=first, stop=last)
                nc.tensor.matmul(sm[64:128, vco:512],
                                 ones_t[:, 0:64],
                                 et[:, 512 + vco:1024],
                                 start=first, stop=last)
                # pace filler groups evenly across the attention steps
                if j % stride == stride - 1 and fillers:
                    fillers.popleft()()
            rc = rc_pool.tile([128, 512], f32, name="recip", tag="recip")
            nc.vector.reciprocal_approx_fast(rc[:], sm[:])
            at = at_pool.tile([128, 512], f32r, name=f"at{hp}", tag=f"at{hp}")
            nc.vector.tensor_mul(at[:], av[:], rc[:])
            at_state[c].append(at)
            if fillers:
                fillers.popleft()()

        # Phase plan: attention units in an order that blends the ACT-heavy
        # late chunks with the PE-heavy qkv/proj filler groups, so the PE
        # never idles long enough to trip the HAM throttle.
        phases = [
            ([(0, 0), (0, 1), (0, 2), (0, 3)], qkv_units(1)),
            ([(1, 0), (1, 1), (1, 2), (1, 3)], qkv_units(2)),
            ([(2, 0), (2, 1), (2, 2), (2, 3)], qkv_units(3)),
            ([(3, 0), (3, 1), (3, 2), (3, 3)],
             proj_units(0) + proj_units(1) + proj_units(2)),
        ]
        for units, filler_list in phases:
            fillers = deque(filler_list)
            total_j = sum(4 * c + 4 for c, hp in units)
            stride = max(1, -(-total_j // max(1, len(filler_list))))
            for (c, hp) in units:
                attn_unit(c, hp, fillers, stride)
            while fillers:
                fillers.popleft()()

        # ---- final chunk's projection (nothing left to hide it behind) ----
        for u in proj_units(NCH - 1):
            u()

    nc.compile()
    return nc


def _get_nc():
    if "nc" not in _CACHE:
        _CACHE["nc"] = _build()
    return _CACHE["nc"]


def _in_maps(x, w_qkv, w_proj):
    wr = w_qkv.reshape(C, 3, H, HD)
    wpr = w_proj.reshape(H, HD, C)
    maps = []
    for core in range(NCORES):
        b, g = core // 2, core % 2
        hs = slice(g * HPC, (g + 1) * HPC)
        maps.append({
            "xT": np.ascontiguousarray(x[b].T),
            "wq": np.ascontiguousarray(wr[:, 0, hs, :].reshape(C, F)),
            "wk": np.ascontiguousarray(wr[:, 1, hs, :].reshape(C, F)),
            "wv": np.ascontiguousarray(wr[:, 2, hs, :].reshape(C, F)),
            "wp": np.ascontiguousarray(wpr[hs].reshape(F, C)),
            "ones": np.ones((128, 64), dtype=ml_dtypes.bfloat16),
        })
    return maps


def kernel(x, w_qkv, w_proj, b_proj, _trace=False):
    from concourse.bass_utils import run_bass_kernel_spmd

    x = np.asarray(x, dtype=np.float32)
    w_qkv = np.asarray(w_qkv, dtype=np.float32)
    w_proj = np.asarray(w_proj, dtype=np.float32)
    b_proj = np.asarray(b_proj, dtype=np.float32)

    nc = _get_nc()
    in_maps = _in_maps(x, w_qkv, w_proj)
    try:
        res = run_bass_kernel_spmd(nc, in_maps, list(range(NCORES)),
                                   trace=_trace)
    except Exception:
        # Device may be wedged from a prior run; reset the axon-side NRT
        # and retry once.
        try:
            import ctypes
            import jax
            lib = ctypes.CDLL("/opt/axon/libaxon_pjrt.so")
            jax.devices()
            lib.axon_reset.restype = ctypes.c_int64
            lib.axon_reset()
        except Exception:
            pass
        res = run_bass_kernel_spmd(nc, in_maps, list(range(NCORES)),
                                   trace=_trace)
    out = np.empty((B, N, C), dtype=np.float32)
    for b in range(B):
        out[b] = res.results[2 * b]["out"] + res.results[2 * b + 1]["out"]
    out += b_proj.reshape(1, 1, C)
    if _trace:
        return out, res
    return out



# revision 2
# speedup vs baseline: 1.1589x; 1.1589x over previous
"""Block-causal attention (B=4, N=2048, C=1024, H=16, block=128) on 8 TRN2 NeuronCores.

Sharding: core = 2*b + g  (b in 0..3 batches, g in 0..1 head-groups of 8 heads).
Each core:
  - computes q^T,k^T (feature-major) and v (token-major) for its batch/head-group
    from a host-pre-transposed x^T and head-sliced w_qkv  (no duplicated FLOPs),
  - block-causal attention: q-tile i attends to k-tiles 0..i (no masking needed,
    128-token blocks align with tiles),
  - partial out-projection with its 512-row slice of w_proj.
Host sums the two partial projections per batch and adds b_proj.

All matmuls run as float32r (single-pass fp22) on the PE.  Attention uses:
  S^T[j] = k_j^T q  (row-packed pair of heads, K=64 each),
  expT = exp(S^T/8) on ScalarE straight out of PSUM,
  out^T += v_j^T expT and row-sums += ones^T expT (col-packed head pairs),
  normalize with DVE reciprocal+mul.  No transposes anywhere on device.
"""

import numpy as np
import ml_dtypes
from collections import deque
from contextlib import ExitStack

B, N, C, H, HD = 4, 2048, 1024, 16, 64
HPC = 8               # heads per core
F = HPC * HD          # 512 features per core
NCORES = 8
SCALE = float(HD) ** -0.5
NT = N // 128         # 16 token tiles
NCH = 4               # token chunks of 512

_CACHE = {}


def _build():
    import concourse.mybir as mybir
    import concourse.tile as tile
    from concourse import bacc

    f32 = mybir.dt.float32
    f32r = mybir.dt.float32r
    bf16 = mybir.dt.bfloat16
    Exp = mybir.ActivationFunctionType.Exp

    nc = bacc.Bacc("TRN2", target_bir_lowering=False, debug=False,
                   num_devices=NCORES)

    xT = nc.dram_tensor("xT", [C, N], f32r, kind="ExternalInput")
    wq = nc.dram_tensor("wq", [C, F], f32r, kind="ExternalInput")
    wk = nc.dram_tensor("wk", [C, F], f32r, kind="ExternalInput")
    wv = nc.dram_tensor("wv", [C, F], f32r, kind="ExternalInput")
    wp = nc.dram_tensor("wp", [F, C], f32r, kind="ExternalInput")
    ones_d = nc.dram_tensor("ones", [128, 64], bf16, kind="ExternalInput")
    out = nc.dram_tensor("out", [N, C], f32, kind="ExternalOutput")

    with tile.TileContext(nc) as tc, ExitStack() as ctx:
        persist = ctx.enter_context(tc.tile_pool(name="persist", bufs=1))
        xt_pool = ctx.enter_context(tc.tile_pool(name="xt", bufs=2))
        qt_pool = ctx.enter_context(tc.tile_pool(name="qt", bufs=2))
        at_pool = ctx.enter_context(tc.tile_pool(name="attnT", bufs=4))
        exp_pool = ctx.enter_context(tc.tile_pool(name="expT", bufs=4))
        rc_pool = ctx.enter_context(tc.tile_pool(name="recip", bufs=1))
        ost_pool = ctx.enter_context(tc.tile_pool(name="ost", bufs=2))
        ps_mm = ctx.enter_context(tc.tile_pool(name="ps_mm", bufs=2, space="PSUM"))
        ps_s = ctx.enter_context(tc.tile_pool(name="ps_s", bufs=2, space="PSUM"))
        ps_av = ctx.enter_context(tc.tile_pool(name="ps_av", bufs=1, space="PSUM"))
        ps_sum = ctx.enter_context(tc.tile_pool(name="ps_sum", bufs=1, space="PSUM"))

        # ---- persistent weights (DMAs emitted later, after chunk-0 xt) ----
        wq_t = [persist.tile([128, F], f32r, name=f"wq{kk}", tag=f"wq{kk}") for kk in range(8)]
        wk_t = [persist.tile([128, F], f32r, name=f"wk{kk}", tag=f"wk{kk}") for kk in range(8)]
        wv_t = [persist.tile([128, F], f32r, name=f"wv{kk}", tag=f"wv{kk}") for kk in range(8)]
        wp_t = [persist.tile([128, C], f32r, name=f"wp{kk}", tag=f"wp{kk}") for kk in range(4)]
        ones_t = persist.tile([128, 64], bf16, name="ones", tag="ones")

        # persistent k^T (per head-pair per chunk) and v (per token tile)
        kt_t = [[persist.tile([128, 512], f32r, name=f"kT{hp}_{jc}", tag=f"kT{hp}_{jc}")
                 for jc in range(NCH)] for hp in range(4)]
        v_t = [persist.tile([128, F], bf16, name=f"v{t}", tag=f"v{t}") for t in range(NT)]

        def load_weights():
            # spread across engine DMA queues so the loads run in parallel
            # with the chunk-0 xt loads on the sync queue
            for kk in range(8):
                nc.scalar.dma_start(wk_t[kk][:], wk[kk * 128:(kk + 1) * 128, :])
            for kk in range(8):
                nc.gpsimd.dma_start(wv_t[kk][:], wv[kk * 128:(kk + 1) * 128, :])
            for kk in range(8):
                nc.sync.dma_start(wq_t[kk][:], wq[kk * 128:(kk + 1) * 128, :])
            nc.gpsimd.dma_start(ones_t[:], ones_d[:])
            for kk in range(4):
                nc.gpsimd.dma_start(wp_t[kk][:], wp[kk * 128:(kk + 1) * 128, :])

        # qt_state[c] / at_state[c] filled lazily by the emit units below
        qt_state = {c: [] for c in range(NCH)}
        at_state = {c: [] for c in range(NCH)}

        def qkv_units(c):
            """Emission units for QKV of token chunk c (xt DMA + 12 matmul
            groups).  Returned as closures so they can be interleaved into
            the previous chunk's attention to keep the PE dense (HAM)."""
            c0 = c * 512
            xt_c = []

            def load():
                for kk in range(8):
                    xt = xt_pool.tile([128, 512], f32r, name=f"xt{kk}",
                                      tag=f"xt{kk}")
                    nc.sync.dma_start(xt[:],
                                      xT[kk * 128:(kk + 1) * 128, c0:c0 + 512])
                    xt_c.append(xt)

            def q_group(hp):
                def emit():
                    ps = ps_mm.tile([128, 512], f32, name="mm", tag="mm")
                    for kk in range(8):
                        nc.tensor.matmul(ps[:],
                                         wq_t[kk][:, hp * 128:(hp + 1) * 128],
                                         xt_c[kk][:],
                                         start=(kk == 0), stop=(kk == 7))
                    qt = qt_pool.tile([128, 512], f32r, name=f"qT{hp}",
                                      tag=f"qT{hp}")
                    nc.vector.tensor_copy(qt[:], ps[:])
                    qt_state[c].append(qt)
                return emit

            def k_group(hp):
                def emit():
                    ps = ps_mm.tile([128, 512], f32, name="mm", tag="mm")
                    for kk in range(8):
                        nc.tensor.matmul(ps[:],
                                         wk_t[kk][:, hp * 128:(hp + 1) * 128],
                                         xt_c[kk][:],
                                         start=(kk == 0), stop=(kk == 7))
                    nc.vector.tensor_copy(kt_t[hp][c][:], ps[:])
                return emit

            def v_group(tl):
                def emit():
                    t = 4 * c + tl
                    ps = ps_mm.tile([128, 512], f32, name="mm", tag="mm")
                    for kk in range(8):
                        nc.tensor.matmul(ps[:],
                                         xt_c[kk][:, tl * 128:(tl + 1) * 128],
                                         wv_t[kk][:],
                                         start=(kk == 0), stop=(kk == 7))
                    nc.vector.tensor_copy(v_t[t][:], ps[:])
                return emit

            units = [load]
            # k/v first (attention chunk c needs them for all j), q last
            for hp in range(4):
                units.append(k_group(hp))
            for tl in range(4):
                units.append(v_group(tl))
            for hp in range(4):
                units.append(q_group(hp))
            return units

        def proj_units(c):
            """Emission units for the partial projection of chunk c."""
            units = []
            for tl in range(4):
                for n2 in range(2):
                    def emit(tl=tl, n2=n2):
                        t = 4 * c + tl
                        ps = ps_mm.tile([128, 512], f32, name="mm", tag="mm")
                        for kk in range(4):
                            nc.tensor.matmul(
                                ps[:],
                                at_state[c][kk][:, tl * 128:(tl + 1) * 128],
                                wp_t[kk][:, n2 * 512:(n2 + 1) * 512],
                                start=(kk == 0), stop=(kk == 3))
                        ost = ost_pool.tile([128, 512], f32, name="ost",
                                            tag="ost")
                        nc.vector.tensor_copy(ost[:], ps[:])
                        nc.sync.dma_start(
                            out[t * 128:(t + 1) * 128,
                                n2 * 512:(n2 + 1) * 512],
                            ost[:])
                    units.append(emit)
            return units

        # chunk 0's QKV has nothing to hide behind — emit it upfront.
        # xt DMAs go first so compute can start before all weights land.
        units0 = qkv_units(0)
        units0[0]()
        load_weights()
        for u in units0[1:]:
            u()

        def attn_unit(c, hp, fillers, stride=2):
            """Attention for (chunk c, head-pair hp): the j-loop over visible
            k-tiles in bursts of 2, with filler matmul groups paced in to keep
            the PE dense.

            Within a burst the PE work is shape-batched — both j's S^T pairs
            (64-row tile mode) back-to-back, then both j's AV+sum pairs
            (64-col mode) — because the PE pays ~240ns of pipeline refill on
            every tile-mode reconfiguration.  Bursting halves the number of
            mode switches, and S^T[j1] naturally overlaps exp[j0] on ScalarE
            so the AV never stalls on the activation."""
            njt = 4 * c + 4
            qt_c = qt_state[c]
            av = ps_av.tile([128, 512], f32, name="av", tag="av")
            sm = ps_sum.tile([128, 512], f32, name="sum", tag="sum")

            def st_exp(j):
                """S^T row-pair + exp for one j; returns the et tile."""
                jd = j - 4 * c
                vco = jd * 128 if jd > 0 else 0
                kt = kt_t[hp][j // 4]
                kc = (j % 4) * 128
                ss = ps_s.tile([128, 1024], f32, name="s", tag="s")
                nc.tensor.matmul(ss[:, vco:512],
                                 kt[0:64, kc:kc + 128],
                                 qt_c[hp][0:64, vco:512],
                                 start=True, stop=True)
                nc.tensor.matmul(ss[:, 512 + vco:1024],
                                 kt[64:128, kc:kc + 128],
                                 qt_c[hp][64:128, vco:512],
                                 start=True, stop=True)
                et = exp_pool.tile([128, 1024], bf16, name="e", tag="e")
                if vco:
                    in3 = ss[:].rearrange("p (b q) -> p b q", b=2)[:, :, vco:512]
                    out3 = et[:].rearrange("p (b q) -> p b q", b=2)[:, :, vco:512]
                    nc.scalar.activation(out3, in3, Exp, scale=SCALE)
                else:
                    nc.scalar.activation(et[:], ss[:], Exp, scale=SCALE)
                return et, (jd * 128 if jd > 0 else 0)

            def av_sum(j, et, vco):
                first, last = (j == 0), (j == njt - 1)
                nc.tensor.matmul(av[0:64, vco:512],
                                 v_t[j][:, hp * 128:hp * 128 + 64],
                                 et[:, vco:512],
                                 start=first, stop=last)
                nc.tensor.matmul(av[64:128, vco:512],
                                 v_t[j][:, hp * 128 + 64:hp * 128 + 128],
                                 et[:, 512 + vco:1024],
                                 start=first, stop=last)
                nc.tensor.matmul(sm[0:64, vco:512],
                                 ones_t[:, 0:64],
                                 et[:, vco:512],
                                 start=first, stop=last)
                nc.tensor.matmul(sm[64:128, vco:512],
                                 ones_t[:, 0:64],
                                 et[:, 512 + vco:1024],
                                 start=first, stop=last)

            for j0 in range(0, njt, 2):
                e0, v0 = st_exp(j0)
                e1, v1 = st_exp(j0 + 1)
                av_sum(j0, e0, v0)
                av_sum(j0 + 1, e1, v1)
                # pace filler groups evenly across the attention steps
                for j in (j0, j0 + 1):
                    if j % stride == stride - 1 and fillers:
                        fillers.popleft()()
            rc = rc_pool.tile([128, 512], f32, name="recip", tag="recip")
            nc.vector.reciprocal_approx_fast(rc[:], sm[:])
            at = at_pool.tile([128, 512], f32r, name=f"at{hp}", tag=f"at{hp}")
            nc.vector.tensor_mul(at[:], av[:], rc[:])
            at_state[c].append(at)
            if fillers:
                fillers.popleft()()

        # Phase plan: attention units in an order that blends the ACT-heavy
        # late chunks with the PE-heavy qkv/proj filler groups, so the PE
        # never idles long enough to trip the HAM throttle.
        phases = [
            ([(0, 0), (0, 1), (0, 2), (0, 3)], qkv_units(1)),
            ([(1, 0), (1, 1), (1, 2), (1, 3)], qkv_units(2)),
            ([(2, 0), (2, 1), (2, 2), (2, 3)], qkv_units(3)),
            ([(3, 0), (3, 1), (3, 2), (3, 3)],
             proj_units(0) + proj_units(1) + proj_units(2)),
        ]
        for units, filler_list in phases:
            fillers = deque(filler_list)
            total_j = sum(4 * c + 4 for c, hp in units)
            stride = max(1, -(-total_j // max(1, len(filler_list))))
            for (c, hp) in units:
                attn_unit(c, hp, fillers, stride)
            while fillers:
                fillers.popleft()()

        # ---- final chunk's projection (nothing left to hide it behind) ----
        for u in proj_units(NCH - 1):
            u()

    nc.compile()
    return nc


def _get_nc():
    if "nc" not in _CACHE:
        _CACHE["nc"] = _build()
    return _CACHE["nc"]


def _in_maps(x, w_qkv, w_proj):
    wr = w_qkv.reshape(C, 3, H, HD)
    wpr = w_proj.reshape(H, HD, C)
    maps = []
    for core in range(NCORES):
        b, g = core // 2, core % 2
        hs = slice(g * HPC, (g + 1) * HPC)
        maps.append({
            "xT": np.ascontiguousarray(x[b].T),
            "wq": np.ascontiguousarray(wr[:, 0, hs, :].reshape(C, F)),
            "wk": np.ascontiguousarray(wr[:, 1, hs, :].reshape(C, F)),
            "wv": np.ascontiguousarray(wr[:, 2, hs, :].reshape(C, F)),
            "wp": np.ascontiguousarray(wpr[hs].reshape(F, C)),
            "ones": np.ones((128, 64), dtype=ml_dtypes.bfloat16),
        })
    return maps


def kernel(x, w_qkv, w_proj, b_proj, _trace=False):
    from concourse.bass_utils import run_bass_kernel_spmd

    x = np.asarray(x, dtype=np.float32)
    w_qkv = np.asarray(w_qkv, dtype=np.float32)
    w_proj = np.asarray(w_proj, dtype=np.float32)
    b_proj = np.asarray(b_proj, dtype=np.float32)

    nc = _get_nc()
    in_maps = _in_maps(x, w_qkv, w_proj)
    try:
        res = run_bass_kernel_spmd(nc, in_maps, list(range(NCORES)),
                                   trace=_trace)
    except Exception:
        # Device may be wedged from a prior run; reset the axon-side NRT
        # and retry once.
        try:
            import ctypes
            import jax
            lib = ctypes.CDLL("/opt/axon/libaxon_pjrt.so")
            jax.devices()
            lib.axon_reset.restype = ctypes.c_int64
            lib.axon_reset()
        except Exception:
            pass
        res = run_bass_kernel_spmd(nc, in_maps, list(range(NCORES)),
                                   trace=_trace)
    out = np.empty((B, N, C), dtype=np.float32)
    for b in range(B):
        out[b] = res.results[2 * b]["out"] + res.results[2 * b + 1]["out"]
    out += b_proj.reshape(1, 1, C)
    if _trace:
        return out, res
    return out

